# revision 15
# baseline (speedup 1.0000x reference)
"""Trainium2 Bass kernel for nn_L2BTAnomalyMapGenerator.

Pipeline (per image):
  1. combined[n] = ||norm(pm_n)-norm(m_n)|| * ||norm(pl_n)-norm(l_n)||
     computed from 6 per-patch reductions over C=768:
       s_xx, s_yy (ACT: fused Square+row-accum), s_xy (DVE: fused
       tensor_tensor_reduce), then combined = sqrt((2-2*c_m)*(2-2*c_l)).
  2. anomaly_map = A @ X32 @ A.T where A = B7^3 @ B5^5 @ R (the bilinear
     resize + all 8 zero-padded box blurs are one exact linear operator).
  3. pred_score = mean of top-200 of the map.

Sharding: pure data parallel, B=16 -> 2 images per core on 8 cores.
"""

import numpy as np

import concourse.bass as bass
import concourse.mybir as mybir
import concourse.tile as tile
from concourse.bass_utils import run_bass_kernel_spmd

# Surface python exceptions from the neuronx_cc compile hook (they are
# otherwise swallowed by the PJRT C++ layer into an opaque INTERNAL error).
try:
    import traceback as _tb

    import libneuronxla as _lnx
    from concourse import bass2jax as _b2j

    _orig_hook = _b2j.neuronx_cc_hook

    def _loud_hook(*args, **kwargs):
        try:
            return _orig_hook(*args, **kwargs)
        except BaseException:
            _tb.print_exc()
            raise

    _b2j.neuronx_cc_hook = _loud_hook
    _b2j.install_neuronx_cc_hook()
    _lnx.neuronx_cc = _loud_hook
except Exception:
    pass

# ---------------------------------------------------------------------------
# Shapes (hardcoded for this problem)
B = 16
N_CORES = 8
B_LOC = B // N_CORES  # 2 images per core
N_PATCH = 1024  # 32x32 patches
C = 768
HP = 32
H = 448
K_TOP = 200  # max(1, int(448*448*0.001))
F32 = mybir.dt.float32

# ---------------------------------------------------------------------------
# Patch: walrus on this stack rejects multiple sync-waits on one instruction;
# Tile's exit drain piles every outstanding sem wait onto a single drain.
# Split them across a chain of drain instructions instead.
from concourse.tile import ScopedClock as _ScopedClock


def _split_drain_and_barrier(self, tick_clock, wait_clock):
    nc = self.nc
    drain_inst = nc.sync.drain()
    wait_clock.add_sem_waits(
        drain_inst.ins, _ScopedClock({None: tick_clock.global_clock})
    )
    si = drain_inst.ins.sync_info
    waits = list(si.on_wait) if si and si.on_wait else []
    if len(waits) > 1:
        drain_inst.ins.sync_info = mybir.SyncInfo(
            on_wait=waits[:1],
            on_update=list(si.on_update) if si.on_update else [],
        )
        for w in waits[1:]:
            extra = nc.sync.drain()
            extra.ins.sync_info = mybir.SyncInfo(on_wait=[w], on_update=[])
    nc.all_engine_barrier()
    popped = nc._tile_sem_poison_stack.pop()
    assert popped is self._sem_poison
    nc.clear_and_free_semaphores(list(self.sems.allocated().values()))
    nc.all_engine_barrier()


tile.TileContext._drain_and_barrier = _split_drain_and_barrier


def _split_multiwaits(nc):
    """Walrus on this stack accepts at most one sync-wait per instruction.
    For any instruction with more, keep one wait and move the rest onto
    single-wait drain carriers inserted just before it (same engine, so the
    engine blocks on each wait in sequence - semantically identical)."""
    uid = 0
    for f in nc.m.functions:
        for bb in f.blocks:
            insts = list(bb.instructions)
            out = []
            changed = False
            for ins in insts:
                si = ins.sync_info
                waits = list(si.on_wait) if si and si.on_wait else []
                if len(waits) > 1:
                    changed = True
                    for w in waits[:-1]:
                        d = mybir.InstDrain(
                            name=f"I-mwsplit-{uid}",
                            engine=ins.engine,
                            sync_info=mybir.SyncInfo(on_wait=[w], on_update=[]),
                        )
                        uid += 1
                        out.append(d)
                    ins.sync_info = mybir.SyncInfo(
                        on_wait=[waits[-1]],
                        on_update=list(si.on_update) if si.on_update else [],
                    )
                out.append(ins)
            if changed:
                bb.instructions = out


# ---------------------------------------------------------------------------
def _build_AT() -> np.ndarray:
    """A.T [32, 448] f32 with A = B7^3 @ B5^5 @ R (exact composite of the
    bilinear 32->448 resize and the 8 zero-padded box blurs)."""

    def resize_matrix(out_n, in_n):
        R = np.zeros((out_n, in_n), dtype=np.float64)
        scale = in_n / out_n
        for i in range(out_n):
            src = (i + 0.5) * scale - 0.5
            f = np.floor(src)
            t = src - f
            i0 = int(np.clip(f, 0, in_n - 1))
            i1 = int(np.clip(f + 1, 0, in_n - 1))
            R[i, i0] += 1.0 - t
            R[i, i1] += t
        return R

    def box_band(n, k, pad):
        Bm = np.zeros((n, n), dtype=np.float64)
        for i in range(n):
            for d in range(-pad, pad + 1):
                j = i + d
                if 0 <= j < n:
                    Bm[i, j] += 1.0 / k
        return Bm

    R = resize_matrix(H, HP)
    B5 = box_band(H, 5, 2)
    B7 = box_band(H, 7, 3)
    M = np.linalg.matrix_power(B7, 3) @ np.linalg.matrix_power(B5, 5)
    A = M @ R  # [448, 32]
    return np.ascontiguousarray(A.T.astype(np.float32))  # [32, 448]


_AT = _build_AT()

# ---------------------------------------------------------------------------
_PROGRAM = None


def _build_program(split_multiwaits=True):
    nc = bass.Bass()
    # inp rows: [m(2048) | l(2048) | pm(2048) | pl(2048)]
    inp = nc.declare_dram_parameter("inp", [4 * B_LOC * N_PATCH, C], F32, isOutput=False)
    at = nc.declare_dram_parameter("at", [HP, H], F32, isOutput=False)
    omap = nc.declare_dram_parameter("omap", [B_LOC * H, H], F32, isOutput=True)

    MULT = mybir.AluOpType.mult
    ADD = mybir.AluOpType.add
    SQUARE = mybir.ActivationFunctionType.Square
    SQRT = mybir.ActivationFunctionType.Sqrt

    with tile.TileContext(nc) as tc:
        with (
            tc.tile_pool(name="consts", bufs=1) as consts,
            tc.tile_pool(name="inp", bufs=3) as inp_p,
            tc.tile_pool(name="junk", bufs=2) as junk,
            tc.tile_pool(name="acc", bufs=2) as accp,
            tc.tile_pool(name="small", bufs=2) as small,
            tc.tile_pool(name="mapp", bufs=2) as mapp,
            tc.tile_pool(name="psum", bufs=2, space="PSUM") as psum,
            tc.tile_pool(name="dram", bufs=2, space="DRAM") as dram,
        ):
            at_sb = consts.tile([HP, H], F32)
            nc.sync.dma_start(out=at_sb[:], in_=at[:])

            omap_v = omap[:].rearrange("(s c p) w -> s p c w", s=B_LOC, c=4)
            # [pair a, row r, half k, c]: a=0 -> (m, pm), a=1 -> (l, pl)
            inp_v = inp[:].rearrange("(k a r) c -> a r k c", k=2, a=2)

            for s in range(B_LOC):
                acc_mm = accp.tile([128, 8], F32, tag="acc_mm")
                acc_ll = accp.tile([128, 8], F32, tag="acc_ll")
                acc_qq = accp.tile([128, 8], F32, tag="acc_qq")
                acc_pp = accp.tile([128, 8], F32, tag="acc_pp")
                acc_mp = accp.tile([128, 8], F32, tag="acc_mp")
                acc_lq = accp.tile([128, 8], F32, tag="acc_lq")

                for t in range(8):
                    r0 = s * N_PATCH + t * 128
                    mp_t = inp_p.tile([128, 2 * C], F32, tag="mp_t")
                    lq_t = inp_p.tile([128, 2 * C], F32, tag="lq_t")
                    nc.sync.dma_start(
                        out=mp_t[:, :C], in_=inp_v[0, r0 : r0 + 128, 0, :]
                    )
                    nc.sync.dma_start(
                        out=mp_t[:, C:], in_=inp_v[0, r0 : r0 + 128, 1, :]
                    )
                    nc.sync.dma_start(
                        out=lq_t[:, :C], in_=inp_v[1, r0 : r0 + 128, 0, :]
                    )
                    nc.sync.dma_start(
                        out=lq_t[:, C:], in_=inp_v[1, r0 : r0 + 128, 1, :]
                    )
                    m_t = mp_t[:, :C]
                    p_t = mp_t[:, C:]
                    l_t = lq_t[:, :C]
                    q_t = lq_t[:, C:]

                    # squares on ACT (fused square + row-accum)
                    ja = junk.tile([128, C], F32, tag="ja")
                    nc.scalar.activation(
                        ja[:], m_t, SQUARE, accum_out=acc_mm[:, t : t + 1]
                    )
                    ja2 = junk.tile([128, C], F32, tag="ja2")
                    nc.scalar.activation(
                        ja2[:], l_t, SQUARE, accum_out=acc_ll[:, t : t + 1]
                    )
                    ja3 = junk.tile([128, C], F32, tag="ja3")
                    if t % 2 == 0:
                        nc.scalar.activation(
                            ja3[:], q_t, SQUARE, accum_out=acc_qq[:, t : t + 1]
                        )
                    else:
                        nc.vector.scalar_tensor_tensor(
                            out=ja3[:], in0=q_t, scalar=1.0, in1=q_t,
                            op0=MULT, op1=MULT, accum_out=acc_qq[:, t : t + 1],
                        )
                    # products on DVE (fused mult + row-accum)
                    jd = junk.tile([128, C], F32, tag="jd")
                    nc.vector.scalar_tensor_tensor(
                        out=jd[:], in0=p_t, scalar=1.0, in1=p_t,
                        op0=MULT, op1=MULT, accum_out=acc_pp[:, t : t + 1],
                    )
                    jd2 = junk.tile([128, C], F32, tag="jd2")
                    nc.vector.scalar_tensor_tensor(
                        out=jd2[:], in0=m_t, scalar=1.0, in1=p_t,
                        op0=MULT, op1=MULT, accum_out=acc_mp[:, t : t + 1],
                    )
                    jd3 = junk.tile([128, C], F32, tag="jd3")
                    nc.vector.scalar_tensor_tensor(
                        out=jd3[:], in0=l_t, scalar=1.0, in1=q_t,
                        op0=MULT, op1=MULT, accum_out=acc_lq[:, t : t + 1],
                    )

                # finish: combined = sqrt((2-2*c_m)*(2-2*c_l))  [128, 8]
                u1 = small.tile([128, 8], F32, tag="u1")
                nc.vector.tensor_mul(u1[:], acc_mm[:], acc_pp[:])
                u2 = small.tile([128, 8], F32, tag="u2")
                nc.vector.tensor_mul(u2[:], acc_ll[:], acc_qq[:])
                su1 = small.tile([128, 8], F32, tag="su1")
                nc.scalar.activation(su1[:], u1[:], SQRT)
                su2 = small.tile([128, 8], F32, tag="su2")
                nc.scalar.activation(su2[:], u2[:], SQRT)
                r1 = small.tile([128, 8], F32, tag="r1")
                nc.vector.reciprocal(r1[:], su1[:])
                r2 = small.tile([128, 8], F32, tag="r2")
                nc.vector.reciprocal(r2[:], su2[:])
                c1 = small.tile([128, 8], F32, tag="c1")
                nc.vector.tensor_mul(c1[:], acc_mp[:], r1[:])
                c2 = small.tile([128, 8], F32, tag="c2")
                nc.vector.tensor_mul(c2[:], acc_lq[:], r2[:])
                a1 = small.tile([128, 8], F32, tag="a1")
                nc.vector.tensor_scalar(a1[:], c1[:], -2.0, 2.0, op0=MULT, op1=ADD)
                a2 = small.tile([128, 8], F32, tag="a2")
                nc.vector.tensor_scalar(a2[:], c2[:], -2.0, 2.0, op0=MULT, op1=ADD)
                cc = small.tile([128, 8], F32, tag="cc")
                nc.vector.tensor_mul(cc[:], a1[:], a2[:])
                comb = small.tile([128, 8], F32, tag="comb")
                nc.scalar.activation(comb[:], cc[:], SQRT)

                # transpose-bounce through DRAM: [128p, 8t] -> flat n=128t+p
                xb = dram.tile([N_PATCH], F32, tag="xb")
                nc.sync.dma_start(
                    out=xb[:].rearrange("(t p) -> p t", p=128), in_=comb[:]
                )
                x_sb = small.tile([HP, HP], F32, tag="x_sb")
                nc.sync.dma_start(
                    out=x_sb[:], in_=xb[:].rearrange("(h w) -> h w", w=HP)
                )

                # map = A @ X @ A.T via T1' = X.T @ A.T ; out = T1'.T @ A.T
                ps1 = psum.tile([HP, H], F32, tag="ps1")
                nc.tensor.matmul(ps1[:], lhsT=x_sb[:], rhs=at_sb[:], start=True, stop=True)
                t1_sb = small.tile([HP, H], F32, tag="t1_sb")
                nc.scalar.copy(t1_sb[:], ps1[:])

                map_sb = mapp.tile([112, 4 * H], F32, tag="map_sb")
                for cch in range(4):
                    ps2 = psum.tile([112, H], F32, tag="ps2")
                    nc.tensor.matmul(
                        ps2[:],
                        lhsT=t1_sb[:, cch * 112 : (cch + 1) * 112],
                        rhs=at_sb[:],
                        start=True,
                        stop=True,
                    )
                    nc.scalar.copy(map_sb[:, cch * H : (cch + 1) * H], ps2[:])

                nc.sync.dma_start(
                    out=omap_v[s],
                    in_=map_sb[:].rearrange("p (c w) -> p c w", c=4),
                )

    if split_multiwaits:
        _split_multiwaits(nc)
    return nc


def _get_program():
    global _PROGRAM
    if _PROGRAM is None:
        _PROGRAM = _build_program()
    return _PROGRAM


# ---------------------------------------------------------------------------
def kernel(
    middle_patch,
    last_patch,
    predicted_middle_patch,
    predicted_last_patch,
    out_h,
    out_w,
):
    assert int(out_h) == H and int(out_w) == H
    m = np.ascontiguousarray(np.asarray(middle_patch, dtype=np.float32))
    l = np.ascontiguousarray(np.asarray(last_patch, dtype=np.float32))
    p = np.ascontiguousarray(np.asarray(predicted_middle_patch, dtype=np.float32))
    q = np.ascontiguousarray(np.asarray(predicted_last_patch, dtype=np.float32))
    assert m.shape == (B, N_PATCH, C)

    nc = _get_program()
    core_ids = list(range(N_CORES))
    in_maps = []
    for i in core_ids:
        sl = slice(i * B_LOC, (i + 1) * B_LOC)
        stacked = np.concatenate(
            [
                m[sl].reshape(B_LOC * N_PATCH, C),
                l[sl].reshape(B_LOC * N_PATCH, C),
                p[sl].reshape(B_LOC * N_PATCH, C),
                q[sl].reshape(B_LOC * N_PATCH, C),
            ],
            axis=0,
        )
        in_maps.append({"inp": stacked, "at": _AT})

    res = run_bass_kernel_spmd(nc, in_maps, core_ids)

    anomaly_map = np.empty((B, 1, H, H), dtype=np.float32)
    for i in core_ids:
        anomaly_map[i * B_LOC : (i + 1) * B_LOC, 0] = res.results[i]["omap"].reshape(
            B_LOC, H, H
        )

    flat = anomaly_map.reshape(B, H * H)
    topk = -np.partition(-flat, K_TOP - 1, axis=1)[:, :K_TOP]
    pred_score = topk.mean(axis=1).astype(np.float32)
    return anomaly_map, pred_score
